# revision 24
# baseline (speedup 1.0000x reference)
"""Trainium2 Bass kernel for nn_CrossAttentionLayer (4-stream cross attention).

kernel(**inputs) takes FULL unsharded inputs (keyed as in setup_inputs) and
returns the full output (tuple of 4 arrays, like the reference). Batch (8) is
sharded 1 element per NeuronCore across 8 cores (pure data parallel).

Geometry per core, with C=512, L=256, H=W=64, N=4096:
  The reference's raw .view on the [L,H,W] conv output re-interprets it as
  [H,W,L]; since L=4*64, token t=(l,b) (l=0..255 conv channel, b=0..15)
  has feature vector y[l, b*256 : (b+1)*256] -- a CONTIGUOUS 256-pixel run
  of row l. Output pixel n = l*16 + b corresponds 1:1 to token (l,b).

  So in the natural [L(part), N(free)] layout, a [128, 256] slice is 128
  tokens x 256 features: attention scalars (sim, softmax, attn) are
  per-partition values -- no cross-partition work anywhere.

v5 design:
  - all HBM traffic narrow: x fp8(e4m3) for the convs, x bf16 for the
    residual, output bf16 (upcast on host). ~40 MiB/core vs 96 fp32.
  - convs as fp8 DoubleRow matmuls (2 contraction rows/partition, weights
    host-scaled x64 to dodge e4m3 subnormals; ACT evacuates with scale 1/64).
  - ctx (attention-weighted v sum) + transpose fused on PE: for each token
    group, matmul(lhsT=v_block, rhs=diag(att[s,s2])) transposes v, applies
    per-token attention, and accumulates over s2 in PSUM. diag tiles are
    built by tensor_scalar_mul(identity, att) on DVE/Pool.
  - PSUM ctxT evacuated (cast fp8) directly into OUTPUT-PIXEL-ordered rhs
    columns (n = l*16 + b), so phase-2 reads rhs contiguously and the
    up-projection also runs as fp8 DoubleRow (contracting both f-halves per
    instruction). Residual added via x64-identity bf16 matmul; evac scale
    1/64 + bias on ACT/DVE.
  - phase-2 residual x tiles prefetched during phase 1 (own pool, bufs=6).
"""

import numpy as np

import concourse.bass as bass
import concourse.bacc as bacc
import concourse.mybir as mybir
from concourse.tile import TileContext
from concourse.bass_utils import run_bass_kernel_spmd

B, C, L, HW = 8, 512, 256, 64
N = HW * HW              # 4096 pixels
F = 256                  # token feature length (= N // 16)
NB = N // F              # 16 b-blocks
EPS = 1e-5
NCORES = 8
CHUNK = 512              # pixel chunk (2 b-blocks)
NCHUNKS = N // CHUNK     # 8
CC = C // 128            # 4 contraction chunks
LT = L // 128            # 2 l-tiles

FP32 = mybir.dt.float32
BF16 = mybir.dt.bfloat16
FP8 = mybir.dt.float8e4
AF = mybir.ActivationFunctionType
ALU = mybir.AluOpType
DR = mybir.MatmulPerfMode.DoubleRow
WSCALE = 64.0            # fp8 weights stored x64 (avoid e4m3 subnormals)

_cached = {}


def _build_program(loop_iters=None):
    nc = bacc.Bacc("TRN2", target_bir_lowering=False, debug=False)

    xs = [nc.declare_dram_parameter(f"x{s}", [C, N], BF16, isOutput=False)
          for s in range(4)]
    x8s = [nc.declare_dram_parameter(f"x8{s}", [C, N], FP8, isOutput=False)
           for s in range(4)]
    # host-prearranged weight images (exact SBUF layouts)
    wkq_d = nc.declare_dram_parameter("wkq", [128, 16 * 2 * 2 * 128], FP8, isOutput=False)
    wd_d = nc.declare_dram_parameter("wd", [128, 8 * 2 * 2 * 128], FP8, isOutput=False)
    wu_d = nc.declare_dram_parameter("wu", [128, 4 * 4 * 2 * 128], FP8, isOutput=False)
    bkq_d = nc.declare_dram_parameter("bkq", [128, 16], FP32, isOutput=False)
    bd_d = nc.declare_dram_parameter("bd", [128, 8], FP32, isOutput=False)
    bu_d = nc.declare_dram_parameter("bu", [128, 16], FP32, isOutput=False)
    idb_d = nc.declare_dram_parameter("identb", [128, 128], BF16, isOutput=False)
    id64_d = nc.declare_dram_parameter("ident64", [128, 128], BF16, isOutput=False)
    os_ = [nc.declare_dram_parameter(f"o{s}", [C, N], BF16, isOutput=True)
           for s in range(4)]

    with TileContext(nc) as tc:
        with (
            tc.tile_pool(name="wpool", bufs=1) as wpool,
            tc.tile_pool(name="xpool", bufs=8) as xpool,
            tc.tile_pool(name="x2pool", bufs=14) as x2pool,
            tc.tile_pool(name="kqvp", bufs=2) as kqvp,
            tc.tile_pool(name="attp", bufs=2) as attp,
            tc.tile_pool(name="rhsp", bufs=1) as rhsp,
            tc.tile_pool(name="dpool", bufs=2) as dpool,
            tc.tile_pool(name="outp", bufs=2) as outp,
            tc.tile_pool(name="ps_c", bufs=2, space="PSUM") as ps_c,
            tc.tile_pool(name="ps_up", bufs=2, space="PSUM") as ps_up,
            tc.tile_pool(name="ps_t", bufs=1, space="PSUM") as ps_t,
        ):
            # ---- weights ----
            wkq = wpool.tile([128, 16, 2, 2, 128], FP8)   # [c, (s,mc), jj, g, m]
            nc.sync.dma_start(out=wkq[:], in_=wkq_d.ap().rearrange(
                "p (a j g m) -> p a j g m", a=16, j=2, g=2))
            wd = wpool.tile([128, 8, 2, 2, 128], FP8)     # [c, (s,lt), jj, g, m]
            nc.sync.dma_start(out=wd[:], in_=wd_d.ap().rearrange(
                "p (a j g m) -> p a j g m", a=8, j=2, g=2))
            wu = wpool.tile([128, 4, 4, 2, 128], FP8)     # [f, s, j, fh, c]
            nc.sync.dma_start(out=wu[:], in_=wu_d.ap().rearrange(
                "p (a j g m) -> p a j g m", a=4, j=4, g=2))
            bkq = wpool.tile([128, 16], FP32)
            nc.sync.dma_start(out=bkq[:], in_=bkq_d.ap())
            bd = wpool.tile([128, 8], FP32)
            nc.sync.dma_start(out=bd[:], in_=bd_d.ap())
            bu = wpool.tile([128, 16], FP32)
            nc.sync.dma_start(out=bu[:], in_=bu_d.ap())
            identb = wpool.tile([128, 128], BF16)
            nc.sync.dma_start(out=identb[:], in_=idb_d.ap())
            ident64 = wpool.tile([128, 128], BF16)
            nc.sync.dma_start(out=ident64[:], in_=id64_d.ap())

            # rhs: transposed ctx in OUTPUT-PIXEL order: col n = l*16 + b
            rhs = rhsp.tile([128, 4, 2, N], FP8)  # [f_local, s, fh, n]

            def emit_convs(ci):
                n0 = ci * CHUNK
                kch, qch, vch = [], [], []
                for s in range(4):
                    xt = xpool.tile([128, 2, 2, CHUNK], FP8, tag="x8", name="xt")
                    nc.sync.dma_start(
                        out=xt[:],
                        in_=x8s[s].ap().rearrange(
                            "(j g p) n -> p j g n", p=128, g=2)[:, :, :, n0:n0 + CHUNK])
                    kc = kqvp.tile([128, LT, CHUNK], BF16, tag=f"k{s}", name=f"kc{s}")
                    qc = kqvp.tile([128, LT, CHUNK], BF16, tag=f"q{s}", name=f"qc{s}")
                    vc = kqvp.tile([128, LT, CHUNK], BF16, tag=f"v{s}", name=f"vc{s}")
                    # k|q: mc 0,1 = k l-tiles; 2,3 = q l-tiles
                    for mc in range(4):
                        pcv = ps_c.tile([128, CHUNK], FP32, tag="conv", name="pcv")
                        for jj in range(2):
                            nc.tensor.matmul(
                                out=pcv[:], lhsT=wkq[:, s * 4 + mc, jj, :, :],
                                rhs=xt[:, jj, :, :], perf_mode=DR,
                                start=(jj == 0), stop=(jj == 1))
                        dst = (kc if mc < 2 else qc)[:, mc % 2, :]
                        nc.scalar.activation(
                            out=dst, in_=pcv[:], func=AF.Relu,
                            bias=bkq[:, s * 4 + mc:s * 4 + mc + 1], scale=1.0 / WSCALE)
                    for mc in range(2):
                        pcv = ps_c.tile([128, CHUNK], FP32, tag="conv", name="pcv2")
                        for jj in range(2):
                            nc.tensor.matmul(
                                out=pcv[:], lhsT=wd[:, s * 2 + mc, jj, :, :],
                                rhs=xt[:, jj, :, :], perf_mode=DR,
                                start=(jj == 0), stop=(jj == 1))
                        nc.scalar.activation(
                            out=vc[:, mc, :], in_=pcv[:], func=AF.Identity,
                            bias=bd[:, s * 2 + mc:s * 2 + mc + 1], scale=1.0 / WSCALE)
                    kch.append(kc)
                    qch.append(qc)
                    vch.append(vc)
                return kch, qch, vch

            def emit_attention(ci, kch, qch, vch):
                # sims[l_local, s, s', lt, b] fp32; softmax has no max-sub
                # (|sim| <~ 2, exp safe in fp32); per-s slices so ACT's exp
                # overlaps the next stream's sims on DVE.
                sims = attp.tile([128, 4, 4, LT, 2], FP32, tag="sims", name="sims")
                ex = attp.tile([128, 4, 4, LT, 2], FP32, tag="ex", name="ex")
                scr = attp.tile([128, F], BF16, tag="scr", name="scr")
                for s in range(4):
                    for s2 in range(4):
                        for lt in range(LT):
                            for b in range(2):
                                nc.vector.scalar_tensor_tensor(
                                    out=scr[:],
                                    in0=qch[s][:, lt, b * F:(b + 1) * F],
                                    scalar=0.0625,
                                    in1=kch[s2][:, lt, b * F:(b + 1) * F],
                                    op0=ALU.mult, op1=ALU.mult,
                                    accum_out=sims[:, s, s2, lt, b:b + 1])
                    nc.scalar.activation(out=ex[:, s], in_=sims[:, s],
                                         func=AF.Exp, bias=0.0, scale=1.0)
                sm = attp.tile([128, 4, LT, 2], FP32, tag="sm", name="sm")
                att = attp.tile([128, 4, 4, LT, 2], FP32, tag="att", name="att")
                for s in range(4):
                    nc.vector.tensor_reduce(
                        out=sm[:, s], in_=ex[:, s].rearrange("p t l b -> p l b t"),
                        axis=mybir.AxisListType.X, op=ALU.add)
                    nc.vector.reciprocal(out=sm[:, s], in_=sm[:, s])
                    nc.vector.tensor_tensor(
                        out=att[:, s], in0=ex[:, s],
                        in1=sm[:, s].rearrange("p l b -> p () l b").broadcast_to(
                            (128, 4, LT, 2)),
                        op=ALU.mult)

                # ---- ctx via PE diag-matmul ----
                for b in range(2):
                    bg = 2 * ci + b   # global b index
                    Ds = []
                    for s in range(4):
                        D = dpool.tile([128, 4, LT, 128], BF16,
                                       tag=f"D{s}", name=f"D{s}{b}")
                        for s2 in range(4):
                            eng = nc.vector if s2 == 0 else nc.gpsimd
                            for lt in range(LT):
                                eng.tensor_scalar_mul(
                                    out=D[:, s2, lt, :], in0=identb[:],
                                    scalar1=att[:, s, s2, lt, b:b + 1])
                        Ds.append(D)
                    psts = [ps_t.tile([128, 2, LT, 128], FP32,
                                      tag=f"pst{s}", name=f"pst{s}{b}")
                            for s in range(4)]
                    for fh in range(2):
                        for lt in range(LT):
                            for s2 in range(4):
                                for s in range(4):
                                    nc.tensor.matmul(
                                        out=psts[s][:, fh, lt, :],
                                        lhsT=vch[s2][:, lt, b * F + fh * 128:
                                                     b * F + (fh + 1) * 128],
                                        rhs=Ds[s][:, s2, lt, :],
                                        start=(fh == 0 and lt == 0 and s2 == 0),
                                        stop=(fh == 1 and lt == 1 and s2 == 3),
                                        skip_group_check=True)
                    # evacuate ctxT into pixel-ordered rhs columns:
                    # psum col (lt, l_local) -> n = (lt*128+l_local)*16 + bg
                    for s in range(4):
                        for fh in range(2):
                            dst = rhs[:, s, fh, :].rearrange(
                                "p (lt ll x) -> p lt ll x",
                                lt=LT, x=NB)[:, :, :, bg:bg + 1]
                            nc.scalar.copy(
                                out=dst,
                                in_=psts[s][:, fh, :, :].rearrange(
                                    "p a b -> p a b ()"))

            def _body():
                # software-pipelined emission: convs of chunk ci+1 are queued
                # on each engine before attention of chunk ci, so in-order
                # engine queues do not convoy across the chunk dependency
                # chain (PE conv -> ACT evac -> DVE attn -> PE diag -> ACT copy)
                def prefetch_x2(p):
                    s, nb = p // NCHUNKS, p % NCHUNKS
                    n0 = nb * CHUNK
                    xt = x2pool.tile([128, CC, CHUNK], BF16, tag="x2", name="xt2")
                    nc.sync.dma_start(
                        out=xt[:],
                        in_=xs[s].ap().rearrange("(j p) n -> p j n", p=128)[:, :, n0:n0 + CHUNK])
                    return xt

                tiles = {}
                xt2s = []
                for ci in range(NCHUNKS + 1):
                    if ci < NCHUNKS:
                        tiles[ci] = emit_convs(ci)
                        # interleave phase-2 residual-x prefetch into SP's queue
                        for _ in range(4):
                            xt2s.append(prefetch_x2(len(xt2s)))
                    if ci >= 1:
                        emit_attention(ci - 1, *tiles.pop(ci - 1))

                # ================= phase 2 =================
                for s in range(4):
                    for nb in range(NCHUNKS):
                        n0 = nb * CHUNK
                        xt = xt2s[s * NCHUNKS + nb]
                        ot = outp.tile([128, CC, CHUNK], BF16, tag="ot", name="ot")
                        for j in range(CC):
                            pup = ps_up.tile([128, CHUNK], FP32, tag="up", name="pup")
                            nc.tensor.matmul(
                                out=pup[:], lhsT=wu[:, s, j, :, :],
                                rhs=rhs[:, s, :, n0:n0 + CHUNK], perf_mode=DR,
                                start=True, stop=False, skip_group_check=True)
                            nc.tensor.matmul(
                                out=pup[:], lhsT=ident64[:],
                                rhs=xt[:, j, :],
                                start=False, stop=True, skip_group_check=True)
                            if j % 2 == 0:
                                nc.vector.tensor_scalar(
                                    out=ot[:, j, :], in0=pup[:],
                                    scalar1=1.0 / WSCALE, op0=ALU.mult,
                                    scalar2=bu[:, s * 4 + j:s * 4 + j + 1],
                                    op1=ALU.add)
                            else:
                                nc.scalar.activation(
                                    out=ot[:, j, :], in_=pup[:], func=AF.Identity,
                                    bias=bu[:, s * 4 + j:s * 4 + j + 1],
                                    scale=1.0 / WSCALE)
                        # out-DMA via gpsimd SWDGE: Pool is idle in
                        # phase 2 and its waits don't convoy ACT/DVE compute
                        nc.gpsimd.dma_start(
                            out=os_[s].ap().rearrange("(j p) n -> p j n", p=128)[:, :, n0:n0 + CHUNK],
                            in_=ot[:])

            if loop_iters is None:
                _body()
            else:
                with tc.For_i(0, loop_iters, 1):
                    _body()

    nc.compile()
    return nc


def _prep_weights(inputs):
    """Fold BN into conv weights host-side; produce exact SBUF images."""
    import ml_dtypes
    f32 = np.float32
    bf16 = ml_dtypes.bfloat16
    fp8 = ml_dtypes.float8_e4m3
    g = {k: np.asarray(v, f32) for k, v in inputs.items()}
    sk = g["gk"] / np.sqrt(g["vk"] + EPS)            # [4, L]
    sq = g["gq"] / np.sqrt(g["vq"] + EPS)
    Wk_f = g["Wk"] * sk[:, :, None]                  # [4, L, C]
    Wq_f = g["Wq"] * sq[:, :, None]
    bk_f = (g["bk"] - g["mk"]) * sk + g["betak"]     # [4, L]
    bq_f = (g["bq"] - g["mq"]) * sq + g["betaq"]

    # wkq image [c_local, (s, mc), jj, g, m]: fp8 DoubleRow lhsT chunks of
    # [Wk_f|Wq_f]^T, scaled by WSCALE; channel c = jj*256 + g*128 + c_local
    wkq = np.zeros((128, 16, 2, 2, 128), f32)
    wdv = np.zeros((128, 8, 2, 2, 128), f32)
    wuv = np.zeros((128, 4, 4, 2, 128), f32)
    for s in range(4):
        Wcat = np.concatenate([Wk_f[s], Wq_f[s]], axis=0)  # [512 (kq-l), C]
        for mc in range(4):
            for jj in range(2):
                for gg in range(2):
                    c0 = jj * 256 + gg * 128
                    wkq[:, s * 4 + mc, jj, gg, :] = WSCALE * \
                        Wcat[mc * 128:(mc + 1) * 128, c0:c0 + 128].T
        for mc in range(2):
            for jj in range(2):
                for gg in range(2):
                    c0 = jj * 256 + gg * 128
                    wdv[:, s * 2 + mc, jj, gg, :] = WSCALE * \
                        g["Wd"][s][mc * 128:(mc + 1) * 128, c0:c0 + 128].T
        # wu DoubleRow image [f_local, s, j, fh, c_local] = WSCALE*WuT[f, c]
        WuT = g["Wu"][s].T                           # [L=256 (f), C]
        for j in range(CC):
            for fh in range(2):
                wuv[:, s, j, fh, :] = WSCALE * \
                    WuT[fh * 128:(fh + 1) * 128, j * 128:(j + 1) * 128]

    bkq = np.zeros((128, 16), f32)
    bdv = np.zeros((128, 8), f32)
    buv = np.zeros((128, 16), f32)
    for s in range(4):
        for mc in range(4):
            src = bk_f[s] if mc < 2 else bq_f[s]
            bkq[:, s * 4 + mc] = src[(mc % 2) * 128:(mc % 2) * 128 + 128]
        for mc in range(2):
            bdv[:, s * 2 + mc] = g["bd"][s][mc * 128:(mc + 1) * 128]
        for j in range(CC):
            buv[:, s * 4 + j] = g["bu"][s][j * 128:(j + 1) * 128]
    ident = np.eye(128, dtype=f32)
    return {
        "wkq": wkq.reshape(128, -1).astype(fp8),
        "wd": wdv.reshape(128, -1).astype(fp8),
        "wu": wuv.reshape(128, -1).astype(fp8),
        "bkq": bkq, "bd": bdv, "bu": buv,
        "identb": ident.astype(bf16),
        "ident64": (WSCALE * ident).astype(bf16),
    }


def get_program(loop_iters=None):
    key = ("nc", loop_iters)
    if key not in _cached:
        _cached[key] = _build_program(loop_iters)
    return _cached[key]


def make_in_maps(inputs):
    import ml_dtypes
    w = _prep_weights(inputs)
    names = ("x_f", "x_g", "x_h", "x_t")
    xf = {nm: np.asarray(inputs[nm], np.float32).reshape(B, C, N) for nm in names}
    xs = {nm: xf[nm].astype(ml_dtypes.bfloat16) for nm in names}
    x8 = {nm: xf[nm].astype(ml_dtypes.float8_e4m3) for nm in names}
    in_maps = []
    for b in range(B):
        m = dict(w)
        for s, nm in enumerate(names):
            m[f"x{s}"] = np.ascontiguousarray(xs[nm][b])
            m[f"x8{s}"] = np.ascontiguousarray(x8[nm][b])
        in_maps.append(m)
    return in_maps


def kernel(**inputs):
    nc = get_program()
    in_maps = make_in_maps(inputs)
    res = run_bass_kernel_spmd(nc, in_maps, core_ids=list(range(NCORES)))
    outs = []
    for s in range(4):
        o = np.stack([np.asarray(res.results[b][f"o{s}"], np.float32)
                      for b in range(B)], axis=0)
        outs.append(o.reshape(B, C, HW, HW))
    return tuple(outs)


# revision 25
# speedup vs baseline: 2.3379x; 2.3379x over previous
"""Trainium2 Bass kernel for nn_CrossAttentionLayer (4-stream cross attention).

kernel(**inputs) takes FULL unsharded inputs (keyed as in setup_inputs) and
returns the full output (tuple of 4 arrays, like the reference). Batch (8) is
sharded 1 element per NeuronCore across 8 cores (pure data parallel).

Geometry per core, with C=512, L=256, H=W=64, N=4096:
  The reference's raw .view on the [L,H,W] conv output re-interprets it as
  [H,W,L]; since L=4*64, token t=(l,b) (l=0..255 conv channel, b=0..15)
  has feature vector y[l, b*256 : (b+1)*256] -- a CONTIGUOUS 256-pixel run
  of row l. Output pixel n = l*16 + b corresponds 1:1 to token (l,b).

  So in the natural [L(part), N(free)] layout, a [128, 256] slice is 128
  tokens x 256 features: attention scalars (sim, softmax, attn) are
  per-partition values -- no cross-partition work anywhere.

v5 design:
  - all HBM traffic narrow: x fp8(e4m3) for the convs, x bf16 for the
    residual, output bf16 (upcast on host). ~40 MiB/core vs 96 fp32.
  - convs as fp8 DoubleRow matmuls (2 contraction rows/partition, weights
    host-scaled x64 to dodge e4m3 subnormals; ACT evacuates with scale 1/64).
  - ctx (attention-weighted v sum) + transpose fused on PE: for each token
    group, matmul(lhsT=v_block, rhs=diag(att[s,s2])) transposes v, applies
    per-token attention, and accumulates over s2 in PSUM. diag tiles are
    built by tensor_scalar_mul(identity, att) on DVE/Pool.
  - PSUM ctxT evacuated (cast fp8) directly into OUTPUT-PIXEL-ordered rhs
    columns (n = l*16 + b), so phase-2 reads rhs contiguously and the
    up-projection also runs as fp8 DoubleRow (contracting both f-halves per
    instruction). Residual added via x64-identity bf16 matmul; evac scale
    1/64 + bias on ACT/DVE.
  - phase-2 residual x tiles prefetched during phase 1 (own pool, bufs=6).
"""

import numpy as np

import concourse.bass as bass
import concourse.bacc as bacc
import concourse.mybir as mybir
from concourse.tile import TileContext
from concourse.bass_utils import run_bass_kernel_spmd

B, C, L, HW = 8, 512, 256, 64
N = HW * HW              # 4096 pixels
F = 256                  # token feature length (= N // 16)
NB = N // F              # 16 b-blocks
EPS = 1e-5
NCORES = 8
CHUNK = 512              # pixel chunk (2 b-blocks)
NCHUNKS = N // CHUNK     # 8
CC = C // 128            # 4 contraction chunks
LT = L // 128            # 2 l-tiles

FP32 = mybir.dt.float32
BF16 = mybir.dt.bfloat16
FP8 = mybir.dt.float8e4
AF = mybir.ActivationFunctionType
ALU = mybir.AluOpType
DR = mybir.MatmulPerfMode.DoubleRow
WSCALE = 64.0            # fp8 weights stored x64 (avoid e4m3 subnormals)

_cached = {}


def _build_program(loop_iters=None):
    nc = bacc.Bacc("TRN2", target_bir_lowering=False, debug=False)

    xs = [nc.declare_dram_parameter(f"x{s}", [C, N], BF16, isOutput=False)
          for s in range(4)]
    x8s = [nc.declare_dram_parameter(f"x8{s}", [C, N], FP8, isOutput=False)
           for s in range(4)]
    # host-prearranged weight images (exact SBUF layouts)
    wkq_d = nc.declare_dram_parameter("wkq", [128, 16 * 2 * 2 * 128], FP8, isOutput=False)
    wd_d = nc.declare_dram_parameter("wd", [128, 8 * 2 * 2 * 128], FP8, isOutput=False)
    wu_d = nc.declare_dram_parameter("wu", [128, 4 * 4 * 2 * 128], FP8, isOutput=False)
    bkq_d = nc.declare_dram_parameter("bkq", [128, 16], FP32, isOutput=False)
    bd_d = nc.declare_dram_parameter("bd", [128, 8], FP32, isOutput=False)
    bu_d = nc.declare_dram_parameter("bu", [128, 16], FP32, isOutput=False)
    idb_d = nc.declare_dram_parameter("identb", [128, 128], BF16, isOutput=False)
    id64_d = nc.declare_dram_parameter("ident64", [128, 128], BF16, isOutput=False)
    os_ = [nc.declare_dram_parameter(f"o{s}", [C, N], BF16, isOutput=True)
           for s in range(4)]

    with TileContext(nc) as tc:
        with (
            tc.tile_pool(name="wpool", bufs=1) as wpool,
            tc.tile_pool(name="xpool", bufs=8) as xpool,
            tc.tile_pool(name="x2pool", bufs=14) as x2pool,
            tc.tile_pool(name="kqvp", bufs=2) as kqvp,
            tc.tile_pool(name="attp", bufs=2) as attp,
            tc.tile_pool(name="rhsp", bufs=1) as rhsp,
            tc.tile_pool(name="dpool", bufs=2) as dpool,
            tc.tile_pool(name="outp", bufs=2) as outp,
            tc.tile_pool(name="ps_c", bufs=2, space="PSUM") as ps_c,
            tc.tile_pool(name="ps_up", bufs=2, space="PSUM") as ps_up,
            tc.tile_pool(name="ps_t", bufs=1, space="PSUM") as ps_t,
        ):
            # ---- weights ----
            wkq = wpool.tile([128, 16, 2, 2, 128], FP8)   # [c, (s,mc), jj, g, m]
            nc.sync.dma_start(out=wkq[:], in_=wkq_d.ap().rearrange(
                "p (a j g m) -> p a j g m", a=16, j=2, g=2))
            wd = wpool.tile([128, 8, 2, 2, 128], FP8)     # [c, (s,lt), jj, g, m]
            nc.sync.dma_start(out=wd[:], in_=wd_d.ap().rearrange(
                "p (a j g m) -> p a j g m", a=8, j=2, g=2))
            wu = wpool.tile([128, 4, 4, 2, 128], FP8)     # [f, s, j, fh, c]
            nc.sync.dma_start(out=wu[:], in_=wu_d.ap().rearrange(
                "p (a j g m) -> p a j g m", a=4, j=4, g=2))
            bkq = wpool.tile([128, 16], FP32)
            nc.sync.dma_start(out=bkq[:], in_=bkq_d.ap())
            bd = wpool.tile([128, 8], FP32)
            nc.sync.dma_start(out=bd[:], in_=bd_d.ap())
            bu = wpool.tile([128, 16], FP32)
            nc.sync.dma_start(out=bu[:], in_=bu_d.ap())
            identb = wpool.tile([128, 128], BF16)
            nc.sync.dma_start(out=identb[:], in_=idb_d.ap())
            ident64 = wpool.tile([128, 128], BF16)
            nc.sync.dma_start(out=ident64[:], in_=id64_d.ap())

            # rhs: transposed ctx in OUTPUT-PIXEL order: col n = l*16 + b
            rhs = rhsp.tile([128, 4, 2, N], FP8)  # [f_local, s, fh, n]

            def emit_convs(ci):
                n0 = ci * CHUNK
                kch, qch, vch = [], [], []
                for s in range(4):
                    xt = xpool.tile([128, 2, 2, CHUNK], FP8, tag="x8", name="xt")
                    nc.sync.dma_start(
                        out=xt[:],
                        in_=x8s[s].ap().rearrange(
                            "(j g p) n -> p j g n", p=128, g=2)[:, :, :, n0:n0 + CHUNK])
                    kc = kqvp.tile([128, LT, CHUNK], BF16, tag=f"k{s}", name=f"kc{s}")
                    qc = kqvp.tile([128, LT, CHUNK], BF16, tag=f"q{s}", name=f"qc{s}")
                    vc = kqvp.tile([128, LT, CHUNK], BF16, tag=f"v{s}", name=f"vc{s}")
                    # k|q: mc 0,1 = k l-tiles; 2,3 = q l-tiles
                    for mc in range(4):
                        pcv = ps_c.tile([128, CHUNK], FP32, tag="conv", name="pcv")
                        for jj in range(2):
                            nc.tensor.matmul(
                                out=pcv[:], lhsT=wkq[:, s * 4 + mc, jj, :, :],
                                rhs=xt[:, jj, :, :], perf_mode=DR,
                                start=(jj == 0), stop=(jj == 1))
                        dst = (kc if mc < 2 else qc)[:, mc % 2, :]
                        nc.scalar.activation(
                            out=dst, in_=pcv[:], func=AF.Relu,
                            bias=bkq[:, s * 4 + mc:s * 4 + mc + 1], scale=1.0 / WSCALE)
                    for mc in range(2):
                        pcv = ps_c.tile([128, CHUNK], FP32, tag="conv", name="pcv2")
                        for jj in range(2):
                            nc.tensor.matmul(
                                out=pcv[:], lhsT=wd[:, s * 2 + mc, jj, :, :],
                                rhs=xt[:, jj, :, :], perf_mode=DR,
                                start=(jj == 0), stop=(jj == 1))
                        nc.scalar.activation(
                            out=vc[:, mc, :], in_=pcv[:], func=AF.Identity,
                            bias=bd[:, s * 2 + mc:s * 2 + mc + 1], scale=1.0 / WSCALE)
                    kch.append(kc)
                    qch.append(qc)
                    vch.append(vc)
                return kch, qch, vch

            def emit_attention(ci, kch, qch, vch):
                # sims[l_local, s, s', lt, b] fp32; softmax has no max-sub
                # (|sim| <~ 2, exp safe in fp32); per-s slices so ACT's exp
                # overlaps the next stream's sims on DVE.
                sims = attp.tile([128, 4, 4, LT, 2], FP32, tag="sims", name="sims")
                ex = attp.tile([128, 4, 4, LT, 2], FP32, tag="ex", name="ex")
                scr = attp.tile([128, F], BF16, tag="scr", name="scr")
                for s in range(4):
                    for s2 in range(4):
                        for lt in range(LT):
                            for b in range(2):
                                nc.vector.scalar_tensor_tensor(
                                    out=scr[:],
                                    in0=qch[s][:, lt, b * F:(b + 1) * F],
                                    scalar=0.0625,
                                    in1=kch[s2][:, lt, b * F:(b + 1) * F],
                                    op0=ALU.mult, op1=ALU.mult,
                                    accum_out=sims[:, s, s2, lt, b:b + 1])
                    nc.scalar.activation(out=ex[:, s], in_=sims[:, s],
                                         func=AF.Exp, bias=0.0, scale=1.0)
                sm = attp.tile([128, 4, LT, 2], FP32, tag="sm", name="sm")
                att = attp.tile([128, 4, 4, LT, 2], FP32, tag="att", name="att")
                for s in range(4):
                    nc.vector.tensor_reduce(
                        out=sm[:, s], in_=ex[:, s].rearrange("p t l b -> p l b t"),
                        axis=mybir.AxisListType.X, op=ALU.add)
                    nc.vector.reciprocal(out=sm[:, s], in_=sm[:, s])
                    nc.vector.tensor_tensor(
                        out=att[:, s], in0=ex[:, s],
                        in1=sm[:, s].rearrange("p l b -> p () l b").broadcast_to(
                            (128, 4, LT, 2)),
                        op=ALU.mult)

                # ---- ctx via PE diag-matmul ----
                for b in range(2):
                    bg = 2 * ci + b   # global b index
                    Ds = []
                    for s in range(4):
                        D = dpool.tile([128, 4, LT, 128], BF16,
                                       tag=f"D{s}", name=f"D{s}{b}")
                        for s2 in range(4):
                            eng = nc.vector
                            for lt in range(LT):
                                eng.tensor_scalar_mul(
                                    out=D[:, s2, lt, :], in0=identb[:],
                                    scalar1=att[:, s, s2, lt, b:b + 1])
                        Ds.append(D)
                    psts = [ps_t.tile([128, 2, LT, 128], FP32,
                                      tag=f"pst{s}", name=f"pst{s}{b}")
                            for s in range(4)]
                    for fh in range(2):
                        for lt in range(LT):
                            for s2 in range(4):
                                for s in range(4):
                                    nc.tensor.matmul(
                                        out=psts[s][:, fh, lt, :],
                                        lhsT=vch[s2][:, lt, b * F + fh * 128:
                                                     b * F + (fh + 1) * 128],
                                        rhs=Ds[s][:, s2, lt, :],
                                        start=(fh == 0 and lt == 0 and s2 == 0),
                                        stop=(fh == 1 and lt == 1 and s2 == 3),
                                        skip_group_check=True)
                    # evacuate ctxT into pixel-ordered rhs columns:
                    # psum col (lt, l_local) -> n = (lt*128+l_local)*16 + bg
                    for s in range(4):
                        for fh in range(2):
                            dst = rhs[:, s, fh, :].rearrange(
                                "p (lt ll x) -> p lt ll x",
                                lt=LT, x=NB)[:, :, :, bg:bg + 1]
                            nc.scalar.copy(
                                out=dst,
                                in_=psts[s][:, fh, :, :].rearrange(
                                    "p a b -> p a b ()"))

            def _body():
                # software-pipelined emission: convs of chunk ci+1 are queued
                # on each engine before attention of chunk ci, so in-order
                # engine queues do not convoy across the chunk dependency
                # chain (PE conv -> ACT evac -> DVE attn -> PE diag -> ACT copy)
                def prefetch_x2(p):
                    s, nb = p // NCHUNKS, p % NCHUNKS
                    n0 = nb * CHUNK
                    xt = x2pool.tile([128, CC, CHUNK], BF16, tag="x2", name="xt2")
                    nc.sync.dma_start(
                        out=xt[:],
                        in_=xs[s].ap().rearrange("(j p) n -> p j n", p=128)[:, :, n0:n0 + CHUNK])
                    return xt

                tiles = {}
                xt2s = []
                for ci in range(NCHUNKS + 1):
                    if ci < NCHUNKS:
                        tiles[ci] = emit_convs(ci)
                        # interleave phase-2 residual-x prefetch into SP's queue
                        for _ in range(4):
                            xt2s.append(prefetch_x2(len(xt2s)))
                    if ci >= 1:
                        emit_attention(ci - 1, *tiles.pop(ci - 1))

                # ================= phase 2 =================
                for s in range(4):
                    for nb in range(NCHUNKS):
                        n0 = nb * CHUNK
                        xt = xt2s[s * NCHUNKS + nb]
                        ot = outp.tile([128, CC, CHUNK], BF16, tag="ot", name="ot")
                        for j in range(CC):
                            pup = ps_up.tile([128, CHUNK], FP32, tag="up", name="pup")
                            nc.tensor.matmul(
                                out=pup[:], lhsT=wu[:, s, j, :, :],
                                rhs=rhs[:, s, :, n0:n0 + CHUNK], perf_mode=DR,
                                start=True, stop=False, skip_group_check=True)
                            nc.tensor.matmul(
                                out=pup[:], lhsT=ident64[:],
                                rhs=xt[:, j, :],
                                start=False, stop=True, skip_group_check=True)
                            if j % 2 == 0:
                                nc.vector.tensor_scalar(
                                    out=ot[:, j, :], in0=pup[:],
                                    scalar1=1.0 / WSCALE, op0=ALU.mult,
                                    scalar2=bu[:, s * 4 + j:s * 4 + j + 1],
                                    op1=ALU.add)
                            else:
                                nc.scalar.activation(
                                    out=ot[:, j, :], in_=pup[:], func=AF.Identity,
                                    bias=bu[:, s * 4 + j:s * 4 + j + 1],
                                    scale=1.0 / WSCALE)
                        nc.scalar.dma_start(
                            out=os_[s].ap().rearrange("(j p) n -> p j n", p=128)[:, :, n0:n0 + CHUNK],
                            in_=ot[:])

            if loop_iters is None:
                _body()
            else:
                with tc.For_i(0, loop_iters, 1):
                    _body()

    nc.compile()
    return nc


def _prep_weights(inputs):
    """Fold BN into conv weights host-side; produce exact SBUF images."""
    import ml_dtypes
    f32 = np.float32
    bf16 = ml_dtypes.bfloat16
    fp8 = ml_dtypes.float8_e4m3
    g = {k: np.asarray(v, f32) for k, v in inputs.items()}
    sk = g["gk"] / np.sqrt(g["vk"] + EPS)            # [4, L]
    sq = g["gq"] / np.sqrt(g["vq"] + EPS)
    Wk_f = g["Wk"] * sk[:, :, None]                  # [4, L, C]
    Wq_f = g["Wq"] * sq[:, :, None]
    bk_f = (g["bk"] - g["mk"]) * sk + g["betak"]     # [4, L]
    bq_f = (g["bq"] - g["mq"]) * sq + g["betaq"]

    # wkq image [c_local, (s, mc), jj, g, m]: fp8 DoubleRow lhsT chunks of
    # [Wk_f|Wq_f]^T, scaled by WSCALE; channel c = jj*256 + g*128 + c_local
    wkq = np.zeros((128, 16, 2, 2, 128), f32)
    wdv = np.zeros((128, 8, 2, 2, 128), f32)
    wuv = np.zeros((128, 4, 4, 2, 128), f32)
    for s in range(4):
        Wcat = np.concatenate([Wk_f[s], Wq_f[s]], axis=0)  # [512 (kq-l), C]
        for mc in range(4):
            for jj in range(2):
                for gg in range(2):
                    c0 = jj * 256 + gg * 128
                    wkq[:, s * 4 + mc, jj, gg, :] = WSCALE * \
                        Wcat[mc * 128:(mc + 1) * 128, c0:c0 + 128].T
        for mc in range(2):
            for jj in range(2):
                for gg in range(2):
                    c0 = jj * 256 + gg * 128
                    wdv[:, s * 2 + mc, jj, gg, :] = WSCALE * \
                        g["Wd"][s][mc * 128:(mc + 1) * 128, c0:c0 + 128].T
        # wu DoubleRow image [f_local, s, j, fh, c_local] = WSCALE*WuT[f, c]
        WuT = g["Wu"][s].T                           # [L=256 (f), C]
        for j in range(CC):
            for fh in range(2):
                wuv[:, s, j, fh, :] = WSCALE * \
                    WuT[fh * 128:(fh + 1) * 128, j * 128:(j + 1) * 128]

    bkq = np.zeros((128, 16), f32)
    bdv = np.zeros((128, 8), f32)
    buv = np.zeros((128, 16), f32)
    for s in range(4):
        for mc in range(4):
            src = bk_f[s] if mc < 2 else bq_f[s]
            bkq[:, s * 4 + mc] = src[(mc % 2) * 128:(mc % 2) * 128 + 128]
        for mc in range(2):
            bdv[:, s * 2 + mc] = g["bd"][s][mc * 128:(mc + 1) * 128]
        for j in range(CC):
            buv[:, s * 4 + j] = g["bu"][s][j * 128:(j + 1) * 128]
    ident = np.eye(128, dtype=f32)
    return {
        "wkq": wkq.reshape(128, -1).astype(fp8),
        "wd": wdv.reshape(128, -1).astype(fp8),
        "wu": wuv.reshape(128, -1).astype(fp8),
        "bkq": bkq, "bd": bdv, "bu": buv,
        "identb": ident.astype(bf16),
        "ident64": (WSCALE * ident).astype(bf16),
    }


def get_program(loop_iters=None):
    key = ("nc", loop_iters)
    if key not in _cached:
        _cached[key] = _build_program(loop_iters)
    return _cached[key]


def make_in_maps(inputs):
    import ml_dtypes
    w = _prep_weights(inputs)
    names = ("x_f", "x_g", "x_h", "x_t")
    xf = {nm: np.asarray(inputs[nm], np.float32).reshape(B, C, N) for nm in names}
    xs = {nm: xf[nm].astype(ml_dtypes.bfloat16) for nm in names}
    x8 = {nm: xf[nm].astype(ml_dtypes.float8_e4m3) for nm in names}
    in_maps = []
    for b in range(B):
        m = dict(w)
        for s, nm in enumerate(names):
            m[f"x{s}"] = np.ascontiguousarray(xs[nm][b])
            m[f"x8{s}"] = np.ascontiguousarray(x8[nm][b])
        in_maps.append(m)
    return in_maps


def kernel(**inputs):
    nc = get_program()
    in_maps = make_in_maps(inputs)
    res = run_bass_kernel_spmd(nc, in_maps, core_ids=list(range(NCORES)))
    outs = []
    for s in range(4):
        o = np.stack([np.asarray(res.results[b][f"o{s}"], np.float32)
                      for b in range(B)], axis=0)
        outs.append(o.reshape(B, C, HW, HW))
    return tuple(outs)
